# revision 39
# baseline (speedup 1.0000x reference)
"""AirportGNN (4-layer GAT, N=50000, E=800000) on 8 TRN2 NeuronCores.

Sharding: nodes 6250/core; edges assigned to the dst-owner core, grouped by
(stream, dst-block) into 128-edge chunks padded to the cross-core max so all
cores run ONE graph. Streams split sources by local row < SPLIT so the table
AllGather can be done in two halves (A while phase A finishes, B overlapped
with stream-A edge processing) and so gather indices fit int16.

Self-loops (PyG fill_value='mean') are host-built edge chunks (one per dst
block, attrs = per-dst mean of incoming edge_attr), so phase B computes the
complete softmax sums and the epilogue has no special self-loop path.

Per layer:
  phase A (per 4-block quad): hT = transpose(h); [hp|a_s|a_d] = hT @ Wcat in
    one PSUM bank per quad; cast/DMA into the local table halves; adw (a_d
    per local node) kept on-chip.  AG_A fires after the A-half quads (mid
    phase B of the same layer), AG_B at the layer end — both overlap phase B.
  phase B (per 32-chunk group): 4x 1024-row dma_gather of 128-col bf16 rows
    (src features + a_s; 2048+ rows per gather crashes the NEFF); alpha =
    a_s[src] + (Mt_fp8 @ adw) + a_e(host-built); exp(leaky) via two exps +
    max; msg scaled in place; segment sums via Ms_fp8 @ msg accumulated per
    dst block in PSUM, drained to stg[96] + den[4].
  epilogue, interleaved per-quad as stream-1 windows complete, two parts
    pipelined one quad apart so the DVE never head-of-line blocks on the
    scalar Exp: part1 = normalize + bias + issue Exp; part2 = ELU (as
    max(x,0)+min(exp x,1); the -1 cancels in LN) + residual + mean/var;
    rstd batched per half-layer (per-quad Sqrt thrashes the ACT table).
Readout: zone rows extracted from local h via per-core one-hot matmuls, tiny
[128,96] AllGather, one-hot reassembly, 3 MLP heads; core 0's output.
"""
import sys
import numpy as np

if '/opt/trn_rl_repo' not in sys.path:
    sys.path.insert(0, '/opt/trn_rl_repo')

import ml_dtypes
import concourse.bacc as bacc
import concourse.mybir as mybir
import concourse.tile as tile
from concourse.ap import AP
from concourse.bass_utils import run_bass_kernel_spmd
from concourse._compat import cdiv

F32 = mybir.dt.float32
BF16 = mybir.dt.bfloat16
FP8 = mybir.dt.float8e4
I16 = mybir.dt.int16
AF = mybir.ActivationFunctionType
OP = mybir.AluOpType
NPF8 = ml_dtypes.float8_e4m3fn
NPBF = ml_dtypes.bfloat16

NCORES = 8
HID, H, C = 96, 4, 24
NEG = 0.2
GRP = 32
LAYERS = 4
ZCAP = 128  # padded zone rows per core


def _bmid(ap, n, axis=1):
    """Insert a 0-step broadcast dim of size n at position axis of an AP."""
    l = [list(x) for x in ap.ap]
    return AP(ap.tensor, ap.offset, l[:axis] + [[0, n]] + l[axis:])


def _btail(ap, n):
    """Replace a trailing size-1 dim with a 0-step broadcast dim of size n."""
    l = [list(x) for x in ap.ap]
    assert l[-1][1] == 1, l
    return AP(ap.tensor, ap.offset, l[:-1] + [[0, n]])


def _wrap_idx(ix, n):
    a = np.zeros((16, cdiv(n, 16)), np.int16)
    ix = np.asarray(ix, np.int64)
    for p in range(16):
        v = ix[p::16]
        a[p, :len(v)] = v.astype(np.int16)
    return np.tile(a, (8, 1))


def host_prep(inputs, N, E):
    NL = N // NCORES
    NBLK = cdiv(NL, 128)
    LASTN = NL - (NBLK - 1) * 128
    NQUAD = cdiv(NBLK, 4)
    SPLIT_BLK = max(1, NBLK // 2)
    SPLIT = min(SPLIT_BLK * 128, NL)
    NLB = NL - SPLIT
    NTA, NTB = NCORES * SPLIT, NCORES * NLB
    assert NTA - 1 <= 32767 and NTB - 1 <= 32767

    x = np.asarray(inputs['x'], np.float32)
    ei = np.asarray(inputs['edge_index'], np.int64)
    ea_np = np.asarray(inputs['edge_attr'], np.float32)
    zone = np.asarray(inputs['zone_idx'], np.int64)
    src_all, dst_all = ei[0], ei[1]
    f32 = lambda k: np.asarray(inputs[k], np.float32)

    # ---- per-core raw edge lists, grouped by (stream, block) ----
    # stream 0: src local row < SPLIT (table A); stream 1: rest (table B)
    raw = []  # raw[k][s][b] = (src_table_idx, dst_rel, ea)
    for k in range(NCORES):
        lo = k * NL
        sel = (dst_all >= lo) & (dst_all < lo + NL)
        es, ed = src_all[sel], dst_all[sel] - lo
        eat = ea_np[sel]
        own = es // NL
        loc = es % NL
        sA = loc < SPLIT
        tix = np.where(sA, own * SPLIT + loc, own * NLB + (loc - SPLIT))
        # self-loops (PyG fill_value='mean') as host-built edges: per-dst mean
        # of incoming edge_attr, dst's own table row as src
        deg = np.bincount(ed, minlength=NL).astype(np.float64)
        la = np.zeros((NL, 4), np.float32)
        for a in range(4):
            la[:, a] = (np.bincount(ed, weights=eat[:, a].astype(np.float64),
                                    minlength=NL)
                        / np.clip(deg, 1.0, None)).astype(np.float32)
        per = [[None] * NBLK for _ in range(2)]
        for s in (0, 1):
            for b in range(NBLK):
                m = (ed // 128 == b) & (sA == (s == 0))
                bs, bd, be = tix[m], ed[m] - b * 128, eat[m]
                o2 = np.argsort(bs, kind='stable')
                per[s][b] = (bs[o2], bd[o2], be[o2])
        for b in range(NBLK):
            nb = min(128, NL - b * 128)
            locs = np.arange(b * 128, b * 128 + nb, dtype=np.int64)
            s = 0 if b < SPLIT_BLK else 1
            tix_self = (k * SPLIT + locs if s == 0
                        else k * NLB + (locs - SPLIT))
            ea_self = la[locs]
            if nb < 128:
                # give padded dst rows a dummy self edge (table row 0, zero
                # attr) so their softmax denominator stays finite — a NaN in
                # those rows would poison the adw matmul contraction
                tix_self = np.concatenate([tix_self, np.zeros(128 - nb, np.int64)])
                ea_self = np.concatenate([ea_self, np.zeros((128 - nb, 4), np.float32)])
            bs0, bd0, be0 = per[s][b]
            per[s][b] = (np.concatenate([tix_self, bs0]),
                         np.concatenate([np.arange(len(tix_self), dtype=bd0.dtype), bd0]),
                         np.concatenate([ea_self, be0]).astype(np.float32))
        raw.append(per)

    ncb = [[max(cdiv(max(len(raw[k][s][b][0]), 1), 128) for k in range(NCORES))
            for b in range(NBLK)] for s in range(2)]
    nch = []
    pads = []
    for s in (0, 1):
        tot = sum(ncb[s])
        pad = (-tot) % GRP
        pads.append(pad)
        nch.append(tot + pad)
    ncht = nch[0] + nch[1]
    NGRP = ncht // GRP
    chunk_win = []
    win_ranges = [[], []]
    for s in (0, 1):
        c = 0
        for b in range(NBLK):
            win_ranges[s].append((c, c + ncb[s][b]))
            chunk_win += [b] * ncb[s][b]
            c += ncb[s][b]
        chunk_win += [-1] * pads[s]  # uniform tail padding: no compute

    # ---- shared weights ----
    W1a = np.concatenate([f32('in_w1'), f32('in_b1')[None, :]], 0)
    W2a = np.concatenate([f32('in_w2'), f32('in_b2')[None, :]], 0)
    Wcat = np.zeros((HID, LAYERS * 104), np.float32)
    w_eh = np.zeros((4, LAYERS, H), np.float32)
    bias_b = np.zeros((128, LAYERS * 96), np.float32)
    lns_b = np.zeros((128, LAYERS * 96), np.float32)
    lnb_b = np.zeros((128, LAYERS * 96), np.float32)
    for l in range(LAYERS):
        W = f32('conv_w')[l]
        As = np.zeros((HID, H), np.float32)
        Ad = np.zeros((HID, H), np.float32)
        for hh in range(H):
            As[hh * C:(hh + 1) * C, hh] = f32('conv_att_src')[l, hh]
            Ad[hh * C:(hh + 1) * C, hh] = f32('conv_att_dst')[l, hh]
        Wcat[:, l * 104:l * 104 + 96] = W
        Wcat[:, l * 104 + 96:l * 104 + 100] = W @ As
        Wcat[:, l * 104 + 100:l * 104 + 104] = W @ Ad
        w_eh[:, l, :] = np.einsum('ahc,hc->ah',
                                  f32('conv_lin_edge')[l].reshape(4, H, C),
                                  f32('conv_att_edge')[l])
        bias_b[:, l * 96:(l + 1) * 96] = f32('conv_bias')[l][None, :]
        lns_b[:, l * 96:(l + 1) * 96] = f32('norm_scale')[l][None, :]
        lnb_b[:, l * 96:(l + 1) * 96] = f32('norm_bias')[l][None, :]

    def head_aug(pre):
        return (np.concatenate([f32(pre + '_w1'), f32(pre + '_b1')[None]], 0),
                np.concatenate([f32(pre + '_w2'), f32(pre + '_b2')[None]], 0),
                np.concatenate([f32(pre + '_w3'), f32(pre + '_b3')[None]], 0))
    heads = [head_aug('cong'), head_aug('delay'), head_aug('jit')]
    ident = np.eye(128, dtype=np.float32)

    # ---- zone prep: per-core owned rows + shared reassembly perms ----
    nz = len(zone)
    NZC = cdiv(nz, 128)
    owner = zone // NL
    owned = [[j for j in range(nz) if owner[j] == k] for k in range(NCORES)]
    assert max(len(o) for o in owned) <= ZCAP
    pasm = np.zeros((NZC, NCORES, 128, 128), np.float32)  # [jc, src core, row, out]
    for k in range(NCORES):
        for r, j in enumerate(owned[k]):
            pasm[j // 128, k, r, j % 128] = 1.0
    pasm = pasm.reshape(NZC * NCORES * 128, 128).astype(NPBF)

    # ---- per-core arrays ----
    in_maps = []
    for k in range(NCORES):
        idx_arr = np.zeros((128, NGRP * GRP * 8), np.int16)
        Mt = np.zeros((128, ncht, 128), NPF8)
        Ms = np.zeros((128, ncht, 128), NPF8)
        ea_chunk = np.zeros((128, ncht, 4), np.float32)
        flat_idx = np.zeros((ncht, 128), np.int64)
        for s in (0, 1):
            off = 0 if s == 0 else nch[0]
            for b in range(NBLK):
                bsrc, bdst, bea = raw[k][s][b]
                c0 = win_ranges[s][b][0] + off
                for i in range(0, len(bsrc), 128):
                    ci = c0 + i // 128
                    n = min(128, len(bsrc) - i)
                    flat_idx[ci, :n] = bsrc[i:i + n]
                    Mt[bdst[i:i + n], ci, np.arange(n)] = 1.0
                    Ms[np.arange(n), ci, bdst[i:i + n]] = 1.0
                    ea_chunk[:n, ci, :] = bea[i:i + n]
        for g in range(NGRP):
            ix = flat_idx[g * GRP:(g + 1) * GRP].reshape(-1)
            idx_arr[:, g * GRP * 8:(g + 1) * GRP * 8] = _wrap_idx(ix, GRP * 128)

        ae = np.einsum('pca,alh->plch', ea_chunk, w_eh).astype(NPBF)

        xa = x[k * NL:(k + 1) * NL]
        xT_aug = np.ascontiguousarray(
            np.concatenate([xa.T, np.ones((1, NL), np.float32)], 0))

        # zone-extraction one-hots: pball[b][p, i]=1 -> out row i = h[p, block b]
        pball = np.zeros((NBLK, 128, ZCAP), np.float32)
        for r, j in enumerate(owned[k]):
            loc = int(zone[j]) - k * NL
            pball[loc // 128, loc % 128, r] = 1.0

        im = {
            'xT_aug': xT_aug, 'idx': idx_arr,
            'Mt': np.ascontiguousarray(Mt.reshape(128, ncht * 128)),
            'Ms': np.ascontiguousarray(Ms.reshape(128, ncht * 128)),
            'ae': np.ascontiguousarray(ae.reshape(128, LAYERS * ncht * 4)),
            'W1a': W1a, 'W2a': W2a, 'Wcat': Wcat,
            'bias_b': bias_b, 'lns_b': lns_b, 'lnb_b': lnb_b,
            'ident': ident,
            'pball': np.ascontiguousarray(pball.reshape(NBLK * 128, ZCAP)),
            'pasm': pasm,
        }
        for hi_, (w1, w2, w3) in enumerate(heads):
            im[f'hw1_{hi_}'], im[f'hw2_{hi_}'], im[f'hw3_{hi_}'] = w1, w2, w3
        in_maps.append(im)

    meta = {
        'N': N, 'NL': NL, 'NBLK': NBLK, 'LASTN': LASTN, 'NZC': NZC,
        'NQUAD': NQUAD, 'SPLIT_BLK': SPLIT_BLK, 'SPLIT': SPLIT, 'NLB': NLB,
        'NTA': NTA, 'NTB': NTB,
        'nch': nch, 'ncht': ncht, 'NGRP': NGRP,
        'chunk_win': chunk_win, 'win_ranges': win_ranges,
        'head_dims': [2, 1, 1],
    }
    return in_maps, meta


def build(meta):
    NL, NBLK, LASTN, NZC = meta['NL'], meta['NBLK'], meta['LASTN'], meta['NZC']
    NQUAD, SPLIT_BLK, SPLIT, NLB = (meta['NQUAD'], meta['SPLIT_BLK'],
                                    meta['SPLIT'], meta['NLB'])
    NTA, NTB = meta['NTA'], meta['NTB']
    nch, ncht, NGRP = meta['nch'], meta['ncht'], meta['NGRP']
    chunk_win, win_ranges = meta['chunk_win'], meta['win_ranges']
    head_dims = meta['head_dims']

    nc = bacc.Bacc('TRN2', target_bir_lowering=False, debug=False, num_swdge_queues=4)
    P = lambda n, s, d, o=False: nc.declare_dram_parameter(n, s, d, isOutput=o)

    xT_aug = P('xT_aug', [13, NL], F32)
    idx_e = P('idx', [128, NGRP * GRP * 8], I16)
    Mt_e = P('Mt', [128, ncht * 128], FP8)
    Ms_e = P('Ms', [128, ncht * 128], FP8)
    ae_e = P('ae', [128, LAYERS * ncht * 4], BF16)
    W1a_e = P('W1a', [13, 96], F32)
    W2a_e = P('W2a', [97, 96], F32)
    Wcat_e = P('Wcat', [HID, LAYERS * 104], F32)
    bias_e = P('bias_b', [128, LAYERS * 96], F32)
    lns_e = P('lns_b', [128, LAYERS * 96], F32)
    lnb_e = P('lnb_b', [128, LAYERS * 96], F32)
    ident_e = P('ident', [128, 128], F32)
    pball_e = P('pball', [NBLK * 128, ZCAP], F32)
    pasm_e = P('pasm', [NZC * NCORES * 128, 128], BF16)
    hw = [(P(f'hw1_{i}', [97, 96], F32), P(f'hw2_{i}', [97, 48], F32),
           P(f'hw3_{i}', [49, head_dims[i]], F32)) for i in range(3)]
    out_e = P('out', [64, 6, 4], F32, o=True)

    tlA = nc.dram_tensor('tlA', [SPLIT, 128], BF16)
    tlB = nc.dram_tensor('tlB', [NLB, 128], BF16)
    tableA = nc.dram_tensor('tableA', [NTA, 128], BF16, addr_space='Shared')
    tableB = nc.dram_tensor('tableB', [NTB, 128], BF16, addr_space='Shared')
    zloc = nc.dram_tensor('zloc', [ZCAP, 96], BF16)
    zall = nc.dram_tensor('zall', [NCORES * ZCAP, 96], BF16, addr_space='Shared')
    rg = [list(range(NCORES))]

    Mt_v = Mt_e[:].rearrange('p (c e) -> p c e', e=128)
    Ms_v = Ms_e[:].rearrange('p (c e) -> p c e', e=128)
    ae_v = ae_e[:].rearrange('p (l c a) -> p l c a', l=LAYERS, a=4)

    with tile.TileContext(nc) as tc:
        with tc.tile_pool(name='const', bufs=1) as cpool, \
             tc.tile_pool(name='big', bufs=1) as bpool, \
             tc.tile_pool(name='st', bufs=1) as spool, \
             tc.tile_pool(name='ps', bufs=1, space='PSUM') as pp:

            def ctile(name, src_ap, shape, dt=F32):
                t = cpool.tile(shape, dt, name=name, tag=name)
                nc.sync.dma_start(t[:], src_ap)
                return t

            ident_t = ctile('ident_t', ident_e[:], [128, 128])
            Wcat_t = ctile('Wcat_t', Wcat_e[:].rearrange('p (l o) -> p l o', l=LAYERS),
                           [HID, LAYERS, 104])
            bias_t = ctile('bias_t', bias_e[:].rearrange('p (l o) -> p l o', l=LAYERS),
                           [128, LAYERS, 96])
            lns_t = ctile('lns_t', lns_e[:].rearrange('p (l o) -> p l o', l=LAYERS),
                          [128, LAYERS, 96])
            lnb_t = ctile('lnb_t', lnb_e[:].rearrange('p (l o) -> p l o', l=LAYERS),
                          [128, LAYERS, 96])

            h_cur = bpool.tile([128, NBLK, 96], F32, name='h0', tag='h', bufs=2)
            gq = [0]  # gather counter: queue i%4 tracks Tile's DMASW sem i%8
            # gather indices + edge attrs are layer-invariant: load once
            idx_all = cpool.tile([128, NGRP * GRP * 8], I16, name='idx_all',
                                 tag='idx_all')
            nc.sync.dma_start(idx_all[:], idx_e[:])
            eps_t = cpool.tile([128, 1], F32, name='eps_t', tag='eps_t')
            nc.vector.memset(eps_t[:], 1e-5)
            adw_t = bpool.tile([128, NBLK, 4], BF16, name='adw_t')
            tbfs = [bpool.tile([128, 4, 128], BF16, name=f'tbf{i}')
                    for i in range(2)]
            for t in tbfs:
                nc.vector.memset(t[:, :, 104:128], 0.0)

            quad_ranges = ([(a, min(a + 4, SPLIT_BLK)) for a in range(0, SPLIT_BLK, 4)]
                           + [(a, min(a + 4, NBLK)) for a in range(SPLIT_BLK, NBLK, 4)])
            identb = cpool.tile([128, 128], BF16, name='identb', tag='identb')
            nc.scalar.activation(identb[:], ident_t[:], AF.Copy)
            Wcat_b = cpool.tile([HID, LAYERS, 104], BF16, name='Wcat_b', tag='Wcat_b')
            nc.scalar.activation(Wcat_b[:], Wcat_t[:], AF.Copy)

            # ---------- input MLP (512-col stripes = 4 node blocks) ----------
            W1a_t = ctile('W1a_t', W1a_e[:], [13, 96])
            W2a_t = ctile('W2a_t', W2a_e[:], [97, 96])
            if LASTN < 128:
                nc.vector.memset(h_cur[:, NBLK - 1, :], 0.0)
            for q in range(NQUAD):
                b0, b1 = q * 4, min(q * 4 + 4, NBLK)
                c0 = b0 * 128
                w = min(512, NL - c0)
                xT_s = spool.tile([13, 512], F32, name='xT_s', tag='xTs', bufs=1)
                nc.sync.dma_start(xT_s[:, 0:w], xT_aug[:, c0:c0 + w])
                ps1 = pp.tile([96, 512], F32, name='ps1', tag='pT', bufs=1)
                nc.tensor.matmul(ps1[:, 0:w], W1a_t[:], xT_s[:, 0:w],
                                 start=True, stop=True)
                tt = spool.tile([97, 512], F32, name='tt', tag='hT', bufs=2)
                nc.scalar.activation(tt[0:96, 0:w], ps1[:, 0:w], AF.Relu)
                nc.vector.memset(tt[96:97, 0:w], 1.0)
                ps2 = pp.tile([128, 4, 96], F32, name='ps2', tag='pA', bufs=1)
                for i, b in enumerate(range(b0, b1)):
                    nb = 128 if b < NBLK - 1 else LASTN
                    nc.tensor.matmul(ps2[0:nb, i, :], tt[:, i * 128:i * 128 + nb],
                                     W2a_t[:], start=True, stop=True)
                if b1 - b0 == 4 and b1 * 128 <= NL:
                    nc.vector.tensor_copy(h_cur[:, b0:b1, :], ps2[:])
                else:
                    for i, b in enumerate(range(b0, b1)):
                        nb = 128 if b < NBLK - 1 else LASTN
                        nc.vector.tensor_copy(h_cur[0:nb, b, :], ps2[0:nb, i, :])

            def phaseA_quad(l, h_src, b0, b1):
                nq = b1 - b0
                hq = spool.tile([128, 4, 96], BF16, name='hq', tag='hq', bufs=2)
                nc.scalar.activation(hq[:, 0:nq, :], h_src[:, b0:b1, :], AF.Copy)
                pt = pp.tile([96, 512], BF16, name='pt', tag='pT', bufs=1)
                for i in range(nq):
                    nc.tensor.transpose(pt[:, i * 128:(i + 1) * 128],
                                        hq[:, i, :], identb[:])
                hT = spool.tile([96, 512], BF16, name='hT', tag='hT', bufs=2)
                nc.scalar.activation(hT[:, 0:nq * 128], pt[:, 0:nq * 128], AF.Copy)
                pa = pp.tile([128, 4, 104], F32, name='pa', tag='pA', bufs=1)
                for i in range(nq):
                    nc.tensor.matmul(pa[:, i, :], hT[:, i * 128:(i + 1) * 128],
                                     Wcat_b[:, l, :], start=True, stop=True)
                nc.vector.tensor_copy(adw_t[:, b0:b1, :], pa[:, 0:nq, 100:104])
                tbf = tbfs[(b0 // 4) % 2]
                nc.scalar.activation(tbf[:, 0:nq, 0:104], pa[:, 0:nq, :], AF.Copy)
                full = b1 < NBLK or LASTN == 128
                if b1 <= SPLIT_BLK:
                    dst = tlA[b0 * 128:b1 * 128, :]
                else:
                    r0 = (b0 - SPLIT_BLK) * 128
                    dst = tlB[r0:r0 + (b1 - b0 - 1) * 128 + (128 if full else LASTN), :]
                if full:
                    nc.sync.dma_start(
                        dst.rearrange('(b p) f -> p b f', p=128), tbf[:, 0:nq, :])
                else:
                    if nq > 1:
                        nc.sync.dma_start(
                            dst[0:(nq - 1) * 128, :].rearrange('(b p) f -> p b f', p=128),
                            tbf[:, 0:nq - 1, :])
                    nc.sync.dma_start(dst[(nq - 1) * 128:, :], tbf[0:LASTN, nq - 1, :])
                if b1 == SPLIT_BLK:
                    nc.gpsimd.collective_compute(
                        'AllGather', OP.bypass, replica_groups=rg,
                        ins=[tlA.ap().opt()], outs=[tableA.ap().opt()])

            def ag_b():
                nc.gpsimd.collective_compute(
                    'AllGather', OP.bypass, replica_groups=rg,
                    ins=[tlB.ap().opt()], outs=[tableB.ap().opt()])

            var_all = bpool.tile([128, NBLK], F32, name='var_all')
            rstd_all = bpool.tile([128, NBLK], F32, name='rstd_all')

            def epilogue_part1(l, sA, den, b0, b1):
                # sA [128, NBLK, 96] holds complete msg sums (self-loops are
                # chunks), den the per-head softmax denominators. ELU is
                # max(x,0)+min(exp(x),1); the -1 shift cancels in LayerNorm.
                nq = b1 - b0
                rec = spool.tile([128, 4, 4], F32, name='rec', tag='rec', bufs=3)
                nc.vector.reciprocal(rec[:, 0:nq, :], den[:, b0:b1, :])
                nc.vector.tensor_tensor(
                    out=sA[:, b0:b1, :].rearrange('p b (h r) -> p b h r', h=4),
                    in0=sA[:, b0:b1, :].rearrange('p b (h r) -> p b h r', h=4),
                    in1=rec[:, 0:nq, :].broadcast_to([128, nq, 4, 24]), op=OP.mult)
                nc.vector.tensor_tensor(out=sA[:, b0:b1, :], in0=sA[:, b0:b1, :],
                                        in1=_bmid(bias_t[:, l, :], nq), op=OP.add)
                emn = spool.tile([128, 4, 96], F32, name='emn', tag='emn', bufs=2)
                nc.scalar.activation(emn[:, 0:nq, :], sA[:, b0:b1, :], AF.Exp)
                return emn

            def epilogue_part2(l, sA, h_cur, emn, b0, b1):
                nq = b1 - b0
                nc.vector.tensor_scalar_min(emn[:, 0:nq, :], emn[:, 0:nq, :], 1.0)
                nc.vector.tensor_scalar_max(sA[:, b0:b1, :], sA[:, b0:b1, :], 0.0)
                nc.vector.tensor_tensor(out=sA[:, b0:b1, :], in0=sA[:, b0:b1, :],
                                        in1=emn[:, 0:nq, :], op=OP.add)
                nc.vector.tensor_tensor(out=sA[:, b0:b1, :], in0=sA[:, b0:b1, :],
                                        in1=h_cur[:, b0:b1, :], op=OP.add)
                mean = spool.tile([128, 4], F32, name='mean', tag='mean', bufs=3)
                nc.vector.tensor_reduce(mean[:, 0:nq], sA[:, b0:b1, :],
                                        axis=mybir.AxisListType.X, op=OP.add)
                nc.vector.tensor_scalar_mul(mean[:, 0:nq], mean[:, 0:nq], 1.0 / 96)
                nc.vector.tensor_tensor(out=sA[:, b0:b1, :], in0=sA[:, b0:b1, :],
                                        in1=mean[:, 0:nq].broadcast_to([128, nq, 96]),
                                        op=OP.subtract)
                sq = spool.tile([128, 4, 96], F32, name='sq', tag='sq', bufs=1)
                nc.vector.tensor_tensor(out=sq[:, 0:nq, :], in0=sA[:, b0:b1, :],
                                        in1=sA[:, b0:b1, :], op=OP.mult)
                nc.vector.tensor_reduce(var_all[:, b0:b1], sq[:, 0:nq, :],
                                        axis=mybir.AxisListType.X, op=OP.add)

            def epilogue_quad2(l, sA, h_new, b0, b1):
                nq = b1 - b0
                nc.vector.tensor_tensor(out=sA[:, b0:b1, :], in0=sA[:, b0:b1, :],
                                        in1=rstd_all[:, b0:b1]
                                        .broadcast_to([128, nq, 96]), op=OP.mult)
                nc.vector.tensor_tensor(out=sA[:, b0:b1, :], in0=sA[:, b0:b1, :],
                                        in1=_bmid(lns_t[:, l, :], nq), op=OP.mult)
                nc.vector.tensor_tensor(out=h_new[:, b0:b1, :], in0=sA[:, b0:b1, :],
                                        in1=_bmid(lnb_t[:, l, :], nq), op=OP.add)

            # ---------- layers ----------
            pending_tail = [None]
            for b0, b1 in quad_ranges:
                phaseA_quad(0, h_cur, b0, b1)
            ag_b()

            for l in range(LAYERS):

                # ---- phase B (epilogue + next layer's phase A interleaved
                # per-quad as stream-1 windows complete, so AG_A/AG_B overlap
                # the remaining phase B work) ----
                stg = bpool.tile([128, NBLK, 96], F32, name=f'stg_{l}', tag='stg0')
                den = bpool.tile([128, NBLK, 4], F32, name=f'den_{l}', tag='den0')
                aeL = bpool.tile([128, ncht, 4], BF16, name=f'ae_{l}', tag='aeL',
                                 bufs=2)
                nc.sync.dma_start(aeL[:], ae_v[:, l, :, :])
                h_new = bpool.tile([128, NBLK, 96], F32, name=f'h{l + 1}',
                                   tag='h', bufs=2)
                next_quad = [0]
                pend = []

                def fire_part2(l=l, stg=stg, h_cur=h_cur, pend=pend):
                    while pend:
                        emn_, a0, a1 = pend.pop(0)
                        epilogue_part2(l, stg, h_cur, emn_, a0, a1)

                def flush_half(hb0, hb1, l=l, stg=stg, h_new=h_new):
                    # one Sqrt per half-layer (per-quad Sqrt thrashes the ACT
                    # table against the Exp used by phase B / ELU)
                    w = hb1 - hb0
                    sdh = spool.tile([128, 32], F32, name='sdh', tag='sdh', bufs=2)
                    nc.scalar.activation(sdh[:, 0:w], var_all[:, hb0:hb1],
                                         AF.Sqrt, bias=eps_t[:, 0:1],
                                         scale=1.0 / 96)
                    nc.vector.reciprocal(rstd_all[:, hb0:hb1], sdh[:, 0:w])
                    for qb0, qb1 in quad_ranges:
                        if qb0 < hb0 or qb1 > hb1:
                            continue
                        epilogue_quad2(l, stg, h_new, qb0, qb1)
                        if l + 1 < LAYERS:
                            phaseA_quad(l + 1, h_new, qb0, qb1)

                def quads_done_through(cb):
                    while (next_quad[0] < len(quad_ranges)
                           and quad_ranges[next_quad[0]][1] <= cb + 1):
                        qb0, qb1 = quad_ranges[next_quad[0]]
                        fire_part2()
                        emn_ = epilogue_part1(l, stg, den, qb0, qb1)
                        pend.append((emn_, qb0, qb1))
                        next_quad[0] += 1
                        if qb1 == SPLIT_BLK:
                            fire_part2()
                            flush_half(0, SPLIT_BLK)

                for s in (0, 1):
                    coff = 0 if s == 0 else nch[0]
                    goff = coff // GRP
                    tbl = tableA if s == 0 else tableB
                    nrows = NTA if s == 0 else NTB
                    cur_ps, cur_b = None, -1
                    for g in range(nch[s] // GRP):
                        cg0 = coff + g * GRP
                        if s == 0 and g == 4 and pending_tail[0] is not None:
                            # previous layer's B-half epilogue + phase A +
                            # AG_B, deferred here so this layer's phase-B ops
                            # lead the DVE queue at the boundary; AG_B still
                            # finishes well before stream 1 needs tableB
                            pending_tail[0]()
                            pending_tail[0] = None
                        gt = spool.tile([128, GRP, 128], BF16, name='gt', tag='gt', bufs=5)
                        for hg in range(GRP // 8):
                            nc.gpsimd.dma_gather(
                                gt[:, hg * 8:(hg + 1) * 8, :], tbl[0:nrows, :],
                                idx_all[:, (goff + g) * GRP * 8 + hg * 64:
                                        (goff + g) * GRP * 8 + (hg + 1) * 64],
                                1024, 1024, 128, queue_num=gq[0] % 4)
                            gq[0] += 1
                        mtt = spool.tile([128, GRP, 128], FP8, name='mtt', tag='mtt', bufs=2)
                        nc.sync.dma_start(mtt[:], Mt_v[:, cg0:cg0 + GRP, :])
                        Mb = spool.tile([128, GRP, 128], FP8, name='Mb', tag='Mb', bufs=2)
                        nc.scalar.dma_start(Mb[:], Ms_v[:, cg0:cg0 + GRP, :])

                        adp = pp.tile([128, GRP * 4], F32, name='adp', tag='pD', bufs=2)
                        npad = sum(1 for c in range(GRP) if chunk_win[cg0 + c] < 0)
                        if npad:
                            nc.vector.memset(adp[:, (GRP - npad) * 4:], 0.0)
                        for c in range(GRP):
                            w = chunk_win[cg0 + c]
                            if w < 0:
                                continue
                            nc.tensor.matmul(adp[:, c * 4:(c + 1) * 4], mtt[:, c, :],
                                             adw_t[:, w, :], start=True, stop=True)
                        alpha = spool.tile([128, GRP, 4], F32, name='alpha', tag='alpha', bufs=2)
                        nc.vector.tensor_tensor(
                            out=alpha[:], in0=gt[:, :, 96:100],
                            in1=adp[:].rearrange('p (c f) -> p c f', c=GRP), op=OP.add)
                        nc.vector.tensor_tensor(out=alpha[:], in0=alpha[:],
                                                in1=aeL[:, cg0:cg0 + GRP, :],
                                                op=OP.add)
                        e1 = spool.tile([128, GRP, 4], F32, name='e1', tag='e1', bufs=2)
                        nc.scalar.activation(e1[:], alpha[:], AF.Exp)
                        e2 = spool.tile([128, GRP, 4], F32, name='e2', tag='e2', bufs=2)
                        nc.scalar.activation(e2[:], alpha[:], AF.Exp, scale=NEG)
                        # ex = max(exp(a), exp(.2a)) = exp(leaky(a)), written
                        # straight into gt cols 96:100 (denominator lane)
                        nc.vector.tensor_tensor(out=gt[:, :, 96:100], in0=e1[:],
                                                in1=e2[:], op=OP.max)
                        nc.vector.tensor_tensor(
                            out=gt[:, :, 0:96].rearrange('p c (h r) -> p c h r', h=4),
                            in0=gt[:, :, 0:96].rearrange('p c (h r) -> p c h r', h=4),
                            in1=_bmid(gt[:, :, 96:100], 24, axis=3), op=OP.mult)
                        for c in range(GRP):
                            cb = chunk_win[cg0 + c]
                            if cb < 0:
                                continue
                            if cb != cur_b:
                                assert cur_ps is None
                                cur_ps = pp.tile([128, 104], F32, name='psb', tag='pB', bufs=3)
                                cur_b = cb
                            first = (cg0 + c) == coff + win_ranges[s][cb][0]
                            last = (cg0 + c) == coff + win_ranges[s][cb][1] - 1
                            nc.tensor.matmul(cur_ps[:, 0:100], Mb[:, c, :], gt[:, c, 0:100],
                                             start=first, stop=last)
                            if last:
                                if s == 0:
                                    nc.scalar.activation(stg[:, cb, :],
                                                         cur_ps[:, 0:96], AF.Copy)
                                    nc.scalar.activation(den[:, cb, :],
                                                         cur_ps[:, 96:100], AF.Copy)
                                else:
                                    nc.vector.tensor_tensor(out=stg[:, cb, :],
                                                            in0=stg[:, cb, :],
                                                            in1=cur_ps[:, 0:96],
                                                            op=OP.add)
                                    nc.vector.tensor_tensor(out=den[:, cb, :],
                                                            in0=den[:, cb, :],
                                                            in1=cur_ps[:, 96:100],
                                                            op=OP.add)
                                    quads_done_through(cb)
                                cur_ps, cur_b = None, -1
                    assert cur_ps is None
                assert next_quad[0] == len(quad_ranges)

                def tail_l(fire=fire_part2, flush=flush_half, ll=l):
                    fire()
                    flush(SPLIT_BLK, NBLK)
                    if ll + 1 < LAYERS:
                        ag_b()
                if l + 1 < LAYERS:
                    pending_tail[0] = tail_l
                else:
                    tail_l()
                h_cur = h_new

            # ---------- readout ----------
            HB = (NBLK + 1) // 2
            pz = pp.tile([ZCAP, 96], F32, name='pz', tag='pT', bufs=1)
            for hf in range(2):
                hb0, hb1 = hf * HB, min(NBLK, (hf + 1) * HB)
                pbt = bpool.tile([128, HB, ZCAP], F32, name=f'pbt{hf}',
                                 tag='pbt', bufs=2)
                nc.sync.dma_start(
                    pbt[:, 0:hb1 - hb0, :],
                    pball_e[hb0 * 128:hb1 * 128, :].rearrange('(b p) z -> p b z', p=128))
                for b in range(hb0, hb1):
                    nc.tensor.matmul(pz[:], pbt[:, b - hb0, :], h_cur[:, b, :],
                                     start=(b == 0), stop=(b == NBLK - 1))
            zlt = spool.tile([ZCAP, 96], BF16, name='zlt', tag='zlt')
            nc.scalar.activation(zlt[:], pz[:], AF.Copy)
            nc.sync.dma_start(zloc.ap(), zlt[:])
            nc.gpsimd.collective_compute(
                'AllGather', OP.bypass, replica_groups=rg,
                ins=[zloc.ap().opt()], outs=[zall.ap().opt()])

            zat = spool.tile([128, NCORES, 96], BF16, name='zat', tag='zat')
            nc.sync.dma_start(
                zat[:], zall.ap().rearrange('(k p) f -> p k f', p=ZCAP))
            z_T = spool.tile([97, NZC * 128], F32, name='z_T', tag='z_T')
            nc.vector.memset(z_T[96:97, :], 1.0)
            for jc in range(NZC):
                pz2 = pp.tile([128, 96], F32, name='pz2', tag='pA', bufs=1)
                pmt = spool.tile([128, NCORES, 128], BF16, name='pmt', tag='pmt', bufs=1)
                nc.scalar.dma_start(
                    pmt[:], pasm_e[jc * NCORES * 128:(jc + 1) * NCORES * 128, :]
                    .rearrange('(k p) z -> p k z', p=128))
                for k in range(NCORES):
                    nc.tensor.matmul(pz2[:], pmt[:, k, :], zat[:, k, :],
                                     start=(k == 0), stop=(k == NCORES - 1))
                zs = spool.tile([128, 96], F32, name='zs', tag='zs', bufs=2)
                nc.vector.tensor_copy(zs[:], pz2[:])
                ptz = pp.tile([96, 128], F32, name='ptz', tag='pD', bufs=2)
                nc.tensor.transpose(ptz[:], zs[:], ident_t[:])
                nc.vector.tensor_copy(z_T[0:96, jc * 128:(jc + 1) * 128], ptz[:])

            outS = spool.tile([128, NZC, 4], F32, name='outS', tag='outS')
            ooff = 0
            for hi_ in range(3):
                o = head_dims[hi_]
                w1t = spool.tile([97, 96], F32, name='w1t', tag='w1t', bufs=2)
                nc.sync.dma_start(w1t[:], hw[hi_][0][:])
                w2t = spool.tile([97, 48], F32, name='w2t', tag='w2t', bufs=2)
                nc.sync.dma_start(w2t[:], hw[hi_][1][:])
                w3t = spool.tile([48, o], F32, name='w3t', tag='w3t', bufs=2)
                nc.sync.dma_start(w3t[:], hw[hi_][2][0:48, :])
                b3t = spool.tile([4, 1], F32, name='b3t', tag='b3t', bufs=2)
                nc.sync.dma_start(b3t[0:o, :], hw[hi_][2][48:49, 0:o].rearrange('a b -> b a'))
                p1 = pp.tile([96, NZC * 128], F32, name='p1', tag='pT', bufs=1)
                nc.tensor.matmul(p1[:], w1t[:], z_T[:], start=True, stop=True)
                t1 = spool.tile([97, NZC * 128], F32, name='t1', tag='t1', bufs=1)
                nc.scalar.activation(t1[0:96, :], p1[:], AF.Relu)
                nc.vector.memset(t1[96:97, :], 1.0)
                p2 = pp.tile([48, NZC * 128], F32, name='p2', tag='pA', bufs=1)
                nc.tensor.matmul(p2[:], w2t[:], t1[:], start=True, stop=True)
                t2 = spool.tile([48, NZC * 128], F32, name='t2', tag='t2', bufs=1)
                nc.scalar.activation(t2[:], p2[:], AF.Relu)
                p3 = pp.tile([4, NZC * 128], F32, name='p3', tag='pD', bufs=2)
                nc.tensor.matmul(p3[0:o, :], w3t[:], t2[:], start=True, stop=True)
                oh = spool.tile([4, NZC * 128], F32, name='oh', tag='oh', bufs=1)
                nc.vector.tensor_scalar(out=oh[0:o, :], in0=p3[0:o, :],
                                        scalar1=b3t[0:o, 0:1], scalar2=None, op0=OP.add)
                for jc in range(NZC):
                    po = pp.tile([128, 4], F32, name='po', tag='pB', bufs=3)
                    nc.tensor.transpose(po[:, 0:o], oh[0:o, jc * 128:(jc + 1) * 128],
                                        ident_t[0:o, 0:o])
                    nc.vector.tensor_copy(outS[:, jc, ooff:ooff + o], po[:, 0:o])
                ooff += o
            nc.sync.dma_start(
                out_e.ap().rearrange('a z f -> (a z) f')
                    .rearrange('(c p) f -> p c f', p=128), outS[:])

    nc.compile()
    return nc


def _run(inputs, trace=False):
    N = int(np.asarray(inputs['x']).shape[0])
    E = int(np.asarray(inputs['edge_index']).shape[1])
    in_maps, meta = host_prep(inputs, N, E)
    nc = build(meta)
    res = run_bass_kernel_spmd(nc, in_maps, core_ids=list(range(NCORES)), trace=trace)
    return np.asarray(res.results[0]['out'], np.float32).reshape(64, 6, 4), res


def kernel(**inputs):
    return _run(inputs, trace=False)[0]



# revision 41
# speedup vs baseline: 1.0445x; 1.0445x over previous
"""AirportGNN (4-layer GAT, N=50000, E=800000) on 8 TRN2 NeuronCores.

Sharding: nodes 6250/core; edges assigned to the dst-owner core, grouped by
(stream, dst-block) into 128-edge chunks padded to the cross-core max so all
cores run ONE graph. Streams split sources by local row < SPLIT so the table
AllGather can be done in two halves (A while phase A finishes, B overlapped
with stream-A edge processing) and so gather indices fit int16.

Self-loops (PyG fill_value='mean') are host-built edge chunks (one per dst
block, attrs = per-dst mean of incoming edge_attr), so phase B computes the
complete softmax sums and the epilogue has no special self-loop path.

Per layer:
  phase A (per 4-block quad): hT = transpose(h); [hp|a_s|a_d] = hT @ Wcat in
    one PSUM bank per quad; cast/DMA into the local table halves; adw (a_d
    per local node) kept on-chip.  AG_A fires after the A-half quads (mid
    phase B of the same layer), AG_B at the layer end — both overlap phase B.
  phase B (per 32-chunk group): 4x 1024-row dma_gather of 128-col bf16 rows
    (src features + a_s; 2048+ rows per gather crashes the NEFF); alpha =
    a_s[src] + (Mt_fp8 @ adw) + a_e(host-built); exp(leaky) via two exps +
    max; msg scaled in place; segment sums via Ms_fp8 @ msg accumulated per
    dst block in PSUM, drained to stg[96] + den[4].
  epilogue, interleaved per-quad as stream-1 windows complete, two parts
    pipelined one quad apart so the DVE never head-of-line blocks on the
    scalar Exp: part1 = normalize + bias + issue Exp; part2 = ELU (as
    max(x,0)+min(exp x,1); the -1 cancels in LN) + residual + mean/var;
    rstd batched per half-layer (per-quad Sqrt thrashes the ACT table).
Readout: zone rows extracted from local h via per-core one-hot matmuls, tiny
[128,96] AllGather, one-hot reassembly, 3 MLP heads; core 0's output.
"""
import sys
import numpy as np

if '/opt/trn_rl_repo' not in sys.path:
    sys.path.insert(0, '/opt/trn_rl_repo')

import ml_dtypes
import concourse.bacc as bacc
import concourse.mybir as mybir
import concourse.tile as tile
from concourse.ap import AP
from concourse.bass_utils import run_bass_kernel_spmd
from concourse._compat import cdiv

F32 = mybir.dt.float32
BF16 = mybir.dt.bfloat16
FP8 = mybir.dt.float8e4
I16 = mybir.dt.int16
AF = mybir.ActivationFunctionType
OP = mybir.AluOpType
NPF8 = ml_dtypes.float8_e4m3fn
NPBF = ml_dtypes.bfloat16

NCORES = 8
HID, H, C = 96, 4, 24
NEG = 0.2
GRP = 32
LAYERS = 4
ZCAP = 128  # padded zone rows per core


def _bmid(ap, n, axis=1):
    """Insert a 0-step broadcast dim of size n at position axis of an AP."""
    l = [list(x) for x in ap.ap]
    return AP(ap.tensor, ap.offset, l[:axis] + [[0, n]] + l[axis:])


def _btail(ap, n):
    """Replace a trailing size-1 dim with a 0-step broadcast dim of size n."""
    l = [list(x) for x in ap.ap]
    assert l[-1][1] == 1, l
    return AP(ap.tensor, ap.offset, l[:-1] + [[0, n]])


def _wrap_idx(ix, n):
    a = np.zeros((16, cdiv(n, 16)), np.int16)
    ix = np.asarray(ix, np.int64)
    for p in range(16):
        v = ix[p::16]
        a[p, :len(v)] = v.astype(np.int16)
    return np.tile(a, (8, 1))


def host_prep(inputs, N, E):
    NL = N // NCORES
    NBLK = cdiv(NL, 128)
    LASTN = NL - (NBLK - 1) * 128
    NQUAD = cdiv(NBLK, 4)
    SPLIT_BLK = max(1, NBLK // 2)
    SPLIT = min(SPLIT_BLK * 128, NL)
    NLB = NL - SPLIT
    NTA, NTB = NCORES * SPLIT, NCORES * NLB
    assert NTA - 1 <= 32767 and NTB - 1 <= 32767

    x = np.asarray(inputs['x'], np.float32)
    ei = np.asarray(inputs['edge_index'], np.int64)
    ea_np = np.asarray(inputs['edge_attr'], np.float32)
    zone = np.asarray(inputs['zone_idx'], np.int64)
    src_all, dst_all = ei[0], ei[1]
    f32 = lambda k: np.asarray(inputs[k], np.float32)

    # ---- per-core raw edge lists, grouped by (stream, block) ----
    # stream 0: src local row < SPLIT (table A); stream 1: rest (table B)
    raw = []  # raw[k][s][b] = (src_table_idx, dst_rel, ea)
    for k in range(NCORES):
        lo = k * NL
        sel = (dst_all >= lo) & (dst_all < lo + NL)
        es, ed = src_all[sel], dst_all[sel] - lo
        eat = ea_np[sel]
        own = es // NL
        loc = es % NL
        sA = loc < SPLIT
        tix = np.where(sA, own * SPLIT + loc, own * NLB + (loc - SPLIT))
        # self-loops (PyG fill_value='mean') as host-built edges: per-dst mean
        # of incoming edge_attr, dst's own table row as src
        deg = np.bincount(ed, minlength=NL).astype(np.float64)
        la = np.zeros((NL, 4), np.float32)
        for a in range(4):
            la[:, a] = (np.bincount(ed, weights=eat[:, a].astype(np.float64),
                                    minlength=NL)
                        / np.clip(deg, 1.0, None)).astype(np.float32)
        per = [[None] * NBLK for _ in range(2)]
        for s in (0, 1):
            for b in range(NBLK):
                m = (ed // 128 == b) & (sA == (s == 0))
                bs, bd, be = tix[m], ed[m] - b * 128, eat[m]
                o2 = np.argsort(bs, kind='stable')
                per[s][b] = (bs[o2], bd[o2], be[o2])
        for b in range(NBLK):
            nb = min(128, NL - b * 128)
            locs = np.arange(b * 128, b * 128 + nb, dtype=np.int64)
            s = 0 if b < SPLIT_BLK else 1
            tix_self = (k * SPLIT + locs if s == 0
                        else k * NLB + (locs - SPLIT))
            ea_self = la[locs]
            if nb < 128:
                # give padded dst rows a dummy self edge (table row 0, zero
                # attr) so their softmax denominator stays finite — a NaN in
                # those rows would poison the adw matmul contraction
                tix_self = np.concatenate([tix_self, np.zeros(128 - nb, np.int64)])
                ea_self = np.concatenate([ea_self, np.zeros((128 - nb, 4), np.float32)])
            bs0, bd0, be0 = per[s][b]
            per[s][b] = (np.concatenate([tix_self, bs0]),
                         np.concatenate([np.arange(len(tix_self), dtype=bd0.dtype), bd0]),
                         np.concatenate([ea_self, be0]).astype(np.float32))
        raw.append(per)

    ncb = [[max(cdiv(max(len(raw[k][s][b][0]), 1), 128) for k in range(NCORES))
            for b in range(NBLK)] for s in range(2)]
    nch = []
    pads = []
    for s in (0, 1):
        tot = sum(ncb[s])
        pad = (-tot) % GRP
        pads.append(pad)
        nch.append(tot + pad)
    ncht = nch[0] + nch[1]
    NGRP = ncht // GRP
    chunk_win = []
    win_ranges = [[], []]
    for s in (0, 1):
        c = 0
        for b in range(NBLK):
            win_ranges[s].append((c, c + ncb[s][b]))
            chunk_win += [b] * ncb[s][b]
            c += ncb[s][b]
        chunk_win += [-1] * pads[s]  # uniform tail padding: no compute

    # ---- shared weights ----
    W1a = np.concatenate([f32('in_w1'), f32('in_b1')[None, :]], 0)
    W2a = np.concatenate([f32('in_w2'), f32('in_b2')[None, :]], 0)
    Wcat = np.zeros((HID, LAYERS * 104), np.float32)
    w_eh = np.zeros((4, LAYERS, H), np.float32)
    bias_b = np.zeros((128, LAYERS * 96), np.float32)
    lns_b = np.zeros((128, LAYERS * 96), np.float32)
    lnb_b = np.zeros((128, LAYERS * 96), np.float32)
    for l in range(LAYERS):
        W = f32('conv_w')[l]
        As = np.zeros((HID, H), np.float32)
        Ad = np.zeros((HID, H), np.float32)
        for hh in range(H):
            As[hh * C:(hh + 1) * C, hh] = f32('conv_att_src')[l, hh]
            Ad[hh * C:(hh + 1) * C, hh] = f32('conv_att_dst')[l, hh]
        Wcat[:, l * 104:l * 104 + 96] = W
        Wcat[:, l * 104 + 96:l * 104 + 100] = W @ As
        Wcat[:, l * 104 + 100:l * 104 + 104] = W @ Ad
        w_eh[:, l, :] = np.einsum('ahc,hc->ah',
                                  f32('conv_lin_edge')[l].reshape(4, H, C),
                                  f32('conv_att_edge')[l])
        bias_b[:, l * 96:(l + 1) * 96] = f32('conv_bias')[l][None, :]
        lns_b[:, l * 96:(l + 1) * 96] = f32('norm_scale')[l][None, :]
        lnb_b[:, l * 96:(l + 1) * 96] = f32('norm_bias')[l][None, :]

    def head_aug(pre):
        return (np.concatenate([f32(pre + '_w1'), f32(pre + '_b1')[None]], 0),
                np.concatenate([f32(pre + '_w2'), f32(pre + '_b2')[None]], 0),
                np.concatenate([f32(pre + '_w3'), f32(pre + '_b3')[None]], 0))
    heads = [head_aug('cong'), head_aug('delay'), head_aug('jit')]
    ident = np.eye(128, dtype=np.float32)

    # ---- zone prep: per-core owned rows + shared reassembly perms ----
    nz = len(zone)
    NZC = cdiv(nz, 128)
    owner = zone // NL
    owned = [[j for j in range(nz) if owner[j] == k] for k in range(NCORES)]
    assert max(len(o) for o in owned) <= ZCAP
    pasm = np.zeros((NZC, NCORES, 128, 128), np.float32)  # [jc, src core, row, out]
    for k in range(NCORES):
        for r, j in enumerate(owned[k]):
            pasm[j // 128, k, r, j % 128] = 1.0
    pasm = pasm.reshape(NZC * NCORES * 128, 128).astype(NPBF)

    # ---- per-core arrays ----
    in_maps = []
    for k in range(NCORES):
        idx_arr = np.zeros((128, NGRP * GRP * 8), np.int16)
        Mt = np.zeros((128, ncht, 128), NPF8)
        Ms = np.zeros((128, ncht, 128), NPF8)
        ea_chunk = np.zeros((128, ncht, 4), np.float32)
        flat_idx = np.zeros((ncht, 128), np.int64)
        for s in (0, 1):
            off = 0 if s == 0 else nch[0]
            for b in range(NBLK):
                bsrc, bdst, bea = raw[k][s][b]
                c0 = win_ranges[s][b][0] + off
                for i in range(0, len(bsrc), 128):
                    ci = c0 + i // 128
                    n = min(128, len(bsrc) - i)
                    flat_idx[ci, :n] = bsrc[i:i + n]
                    Mt[bdst[i:i + n], ci, np.arange(n)] = 1.0
                    Ms[np.arange(n), ci, bdst[i:i + n]] = 1.0
                    ea_chunk[:n, ci, :] = bea[i:i + n]
        for g in range(NGRP):
            ix = flat_idx[g * GRP:(g + 1) * GRP].reshape(-1)
            idx_arr[:, g * GRP * 8:(g + 1) * GRP * 8] = _wrap_idx(ix, GRP * 128)

        ae = np.einsum('pca,alh->plch', ea_chunk, w_eh).astype(NPBF)

        xa = x[k * NL:(k + 1) * NL]
        xT_aug = np.ascontiguousarray(
            np.concatenate([xa.T, np.ones((1, NL), np.float32)], 0))

        # zone-extraction one-hots: pball[b][p, i]=1 -> out row i = h[p, block b]
        pball = np.zeros((NBLK, 128, ZCAP), np.float32)
        for r, j in enumerate(owned[k]):
            loc = int(zone[j]) - k * NL
            pball[loc // 128, loc % 128, r] = 1.0

        im = {
            'xT_aug': xT_aug, 'idx': idx_arr,
            'Mt': np.ascontiguousarray(Mt.reshape(128, ncht * 128)),
            'Ms': np.ascontiguousarray(Ms.reshape(128, ncht * 128)),
            'ae': np.ascontiguousarray(ae.reshape(128, LAYERS * ncht * 4)),
            'W1a': W1a, 'W2a': W2a, 'Wcat': Wcat,
            'bias_b': bias_b, 'lns_b': lns_b, 'lnb_b': lnb_b,
            'ident': ident,
            'pball': np.ascontiguousarray(pball.reshape(NBLK * 128, ZCAP)),
            'pasm': pasm,
        }
        for hi_, (w1, w2, w3) in enumerate(heads):
            im[f'hw1_{hi_}'], im[f'hw2_{hi_}'], im[f'hw3_{hi_}'] = w1, w2, w3
        in_maps.append(im)

    meta = {
        'N': N, 'NL': NL, 'NBLK': NBLK, 'LASTN': LASTN, 'NZC': NZC,
        'NQUAD': NQUAD, 'SPLIT_BLK': SPLIT_BLK, 'SPLIT': SPLIT, 'NLB': NLB,
        'NTA': NTA, 'NTB': NTB,
        'nch': nch, 'ncht': ncht, 'NGRP': NGRP,
        'chunk_win': chunk_win, 'win_ranges': win_ranges,
        'head_dims': [2, 1, 1],
    }
    return in_maps, meta


def build(meta):
    NL, NBLK, LASTN, NZC = meta['NL'], meta['NBLK'], meta['LASTN'], meta['NZC']
    NQUAD, SPLIT_BLK, SPLIT, NLB = (meta['NQUAD'], meta['SPLIT_BLK'],
                                    meta['SPLIT'], meta['NLB'])
    NTA, NTB = meta['NTA'], meta['NTB']
    nch, ncht, NGRP = meta['nch'], meta['ncht'], meta['NGRP']
    chunk_win, win_ranges = meta['chunk_win'], meta['win_ranges']
    head_dims = meta['head_dims']

    nc = bacc.Bacc('TRN2', target_bir_lowering=False, debug=False, num_swdge_queues=4)
    P = lambda n, s, d, o=False: nc.declare_dram_parameter(n, s, d, isOutput=o)

    xT_aug = P('xT_aug', [13, NL], F32)
    idx_e = P('idx', [128, NGRP * GRP * 8], I16)
    Mt_e = P('Mt', [128, ncht * 128], FP8)
    Ms_e = P('Ms', [128, ncht * 128], FP8)
    ae_e = P('ae', [128, LAYERS * ncht * 4], BF16)
    W1a_e = P('W1a', [13, 96], F32)
    W2a_e = P('W2a', [97, 96], F32)
    Wcat_e = P('Wcat', [HID, LAYERS * 104], F32)
    bias_e = P('bias_b', [128, LAYERS * 96], F32)
    lns_e = P('lns_b', [128, LAYERS * 96], F32)
    lnb_e = P('lnb_b', [128, LAYERS * 96], F32)
    ident_e = P('ident', [128, 128], F32)
    pball_e = P('pball', [NBLK * 128, ZCAP], F32)
    pasm_e = P('pasm', [NZC * NCORES * 128, 128], BF16)
    hw = [(P(f'hw1_{i}', [97, 96], F32), P(f'hw2_{i}', [97, 48], F32),
           P(f'hw3_{i}', [49, head_dims[i]], F32)) for i in range(3)]
    out_e = P('out', [64, 6, 4], F32, o=True)

    tlA = nc.dram_tensor('tlA', [SPLIT, 128], BF16)
    tlB = nc.dram_tensor('tlB', [NLB, 128], BF16)
    tableA = nc.dram_tensor('tableA', [NTA, 128], BF16, addr_space='Shared')
    tableB = nc.dram_tensor('tableB', [NTB, 128], BF16, addr_space='Shared')
    zloc = nc.dram_tensor('zloc', [ZCAP, 96], BF16)
    zall = nc.dram_tensor('zall', [NCORES * ZCAP, 96], BF16, addr_space='Shared')
    rg = [list(range(NCORES))]

    Mt_v = Mt_e[:].rearrange('p (c e) -> p c e', e=128)
    Ms_v = Ms_e[:].rearrange('p (c e) -> p c e', e=128)
    ae_v = ae_e[:].rearrange('p (l c a) -> p l c a', l=LAYERS, a=4)

    with tile.TileContext(nc) as tc:
        with tc.tile_pool(name='const', bufs=1) as cpool, \
             tc.tile_pool(name='big', bufs=1) as bpool, \
             tc.tile_pool(name='st', bufs=1) as spool, \
             tc.tile_pool(name='ps', bufs=1, space='PSUM') as pp:

            def ctile(name, src_ap, shape, dt=F32):
                t = cpool.tile(shape, dt, name=name, tag=name)
                nc.sync.dma_start(t[:], src_ap)
                return t

            ident_t = ctile('ident_t', ident_e[:], [128, 128])
            Wcat_t = ctile('Wcat_t', Wcat_e[:].rearrange('p (l o) -> p l o', l=LAYERS),
                           [HID, LAYERS, 104])
            bias_t = ctile('bias_t', bias_e[:].rearrange('p (l o) -> p l o', l=LAYERS),
                           [128, LAYERS, 96])
            lns_t = ctile('lns_t', lns_e[:].rearrange('p (l o) -> p l o', l=LAYERS),
                          [128, LAYERS, 96])
            lnb_t = ctile('lnb_t', lnb_e[:].rearrange('p (l o) -> p l o', l=LAYERS),
                          [128, LAYERS, 96])

            h_cur = bpool.tile([128, NBLK, 96], F32, name='h0', tag='h', bufs=2)
            gq = [0]  # gather counter: queue i%4 tracks Tile's DMASW sem i%8
            # gather indices + edge attrs are layer-invariant: load once
            idx_all = cpool.tile([128, NGRP * GRP * 8], I16, name='idx_all',
                                 tag='idx_all')
            nc.sync.dma_start(idx_all[:], idx_e[:])
            eps_t = cpool.tile([128, 1], F32, name='eps_t', tag='eps_t')
            nc.vector.memset(eps_t[:], 1e-5)
            adw_t = bpool.tile([128, NBLK, 4], BF16, name='adw_t')
            tbfs = [bpool.tile([128, 4, 128], BF16, name=f'tbf{i}')
                    for i in range(2)]
            for t in tbfs:
                nc.vector.memset(t[:, :, 104:128], 0.0)

            quad_ranges = ([(a, min(a + 4, SPLIT_BLK)) for a in range(0, SPLIT_BLK, 4)]
                           + [(a, min(a + 4, NBLK)) for a in range(SPLIT_BLK, NBLK, 4)])
            identb = cpool.tile([128, 128], BF16, name='identb', tag='identb')
            nc.scalar.activation(identb[:], ident_t[:], AF.Copy)
            Wcat_b = cpool.tile([HID, LAYERS, 104], BF16, name='Wcat_b', tag='Wcat_b')
            nc.scalar.activation(Wcat_b[:], Wcat_t[:], AF.Copy)

            # ---------- input MLP (512-col stripes = 4 node blocks) ----------
            W1a_t = ctile('W1a_t', W1a_e[:], [13, 96])
            W2a_t = ctile('W2a_t', W2a_e[:], [97, 96])
            if LASTN < 128:
                nc.vector.memset(h_cur[:, NBLK - 1, :], 0.0)
            for q in range(NQUAD):
                b0, b1 = q * 4, min(q * 4 + 4, NBLK)
                c0 = b0 * 128
                w = min(512, NL - c0)
                xT_s = spool.tile([13, 512], F32, name='xT_s', tag='xTs', bufs=1)
                nc.sync.dma_start(xT_s[:, 0:w], xT_aug[:, c0:c0 + w])
                ps1 = pp.tile([96, 512], F32, name='ps1', tag='pT', bufs=1)
                nc.tensor.matmul(ps1[:, 0:w], W1a_t[:], xT_s[:, 0:w],
                                 start=True, stop=True)
                tt = spool.tile([97, 512], F32, name='tt', tag='hT', bufs=2)
                nc.scalar.activation(tt[0:96, 0:w], ps1[:, 0:w], AF.Relu)
                nc.vector.memset(tt[96:97, 0:w], 1.0)
                ps2 = pp.tile([128, 4, 96], F32, name='ps2', tag='pA', bufs=1)
                for i, b in enumerate(range(b0, b1)):
                    nb = 128 if b < NBLK - 1 else LASTN
                    nc.tensor.matmul(ps2[0:nb, i, :], tt[:, i * 128:i * 128 + nb],
                                     W2a_t[:], start=True, stop=True)
                if b1 - b0 == 4 and b1 * 128 <= NL:
                    nc.vector.tensor_copy(h_cur[:, b0:b1, :], ps2[:])
                else:
                    for i, b in enumerate(range(b0, b1)):
                        nb = 128 if b < NBLK - 1 else LASTN
                        nc.vector.tensor_copy(h_cur[0:nb, b, :], ps2[0:nb, i, :])

            def phaseA_quad(l, h_src, b0, b1):
                nq = b1 - b0
                hq = spool.tile([128, 4, 96], BF16, name='hq', tag='hq', bufs=2)
                nc.scalar.activation(hq[:, 0:nq, :], h_src[:, b0:b1, :], AF.Copy)
                pt = pp.tile([96, 512], BF16, name='pt', tag='pT', bufs=1)
                for i in range(nq):
                    nc.tensor.transpose(pt[:, i * 128:(i + 1) * 128],
                                        hq[:, i, :], identb[:])
                hT = spool.tile([96, 512], BF16, name='hT', tag='hT', bufs=2)
                nc.scalar.activation(hT[:, 0:nq * 128], pt[:, 0:nq * 128], AF.Copy)
                pa = pp.tile([128, 4, 104], F32, name='pa', tag='pA', bufs=1)
                for i in range(nq):
                    nc.tensor.matmul(pa[:, i, :], hT[:, i * 128:(i + 1) * 128],
                                     Wcat_b[:, l, :], start=True, stop=True)
                nc.vector.tensor_copy(adw_t[:, b0:b1, :], pa[:, 0:nq, 100:104])
                tbf = tbfs[(b0 // 4) % 2]
                nc.scalar.activation(tbf[:, 0:nq, 0:104], pa[:, 0:nq, :], AF.Copy)
                full = b1 < NBLK or LASTN == 128
                if b1 <= SPLIT_BLK:
                    dst = tlA[b0 * 128:b1 * 128, :]
                else:
                    r0 = (b0 - SPLIT_BLK) * 128
                    dst = tlB[r0:r0 + (b1 - b0 - 1) * 128 + (128 if full else LASTN), :]
                if full:
                    nc.sync.dma_start(
                        dst.rearrange('(b p) f -> p b f', p=128), tbf[:, 0:nq, :])
                else:
                    if nq > 1:
                        nc.sync.dma_start(
                            dst[0:(nq - 1) * 128, :].rearrange('(b p) f -> p b f', p=128),
                            tbf[:, 0:nq - 1, :])
                    nc.sync.dma_start(dst[(nq - 1) * 128:, :], tbf[0:LASTN, nq - 1, :])
                if b1 == SPLIT_BLK:
                    nc.gpsimd.collective_compute(
                        'AllGather', OP.bypass, replica_groups=rg,
                        ins=[tlA.ap().opt()], outs=[tableA.ap().opt()])

            def ag_b():
                nc.gpsimd.collective_compute(
                    'AllGather', OP.bypass, replica_groups=rg,
                    ins=[tlB.ap().opt()], outs=[tableB.ap().opt()])

            var_all = bpool.tile([128, NBLK], F32, name='var_all')
            rstd_all = bpool.tile([128, NBLK], F32, name='rstd_all')

            def epilogue_part1(l, sA, den, b0, b1):
                # sA [128, NBLK, 96] holds complete msg sums (self-loops are
                # chunks), den the per-head softmax denominators. ELU is
                # max(x,0)+min(exp(x),1); the -1 shift cancels in LayerNorm.
                nq = b1 - b0
                rec = spool.tile([128, 4, 4], F32, name='rec', tag='rec', bufs=3)
                nc.vector.reciprocal(rec[:, 0:nq, :], den[:, b0:b1, :])
                nc.vector.tensor_tensor(
                    out=sA[:, b0:b1, :].rearrange('p b (h r) -> p b h r', h=4),
                    in0=sA[:, b0:b1, :].rearrange('p b (h r) -> p b h r', h=4),
                    in1=rec[:, 0:nq, :].broadcast_to([128, nq, 4, 24]), op=OP.mult)
                nc.vector.tensor_tensor(out=sA[:, b0:b1, :], in0=sA[:, b0:b1, :],
                                        in1=_bmid(bias_t[:, l, :], nq), op=OP.add)
                emn = spool.tile([128, 4, 96], F32, name='emn', tag='emn', bufs=2)
                nc.scalar.activation(emn[:, 0:nq, :], sA[:, b0:b1, :], AF.Exp)
                return emn

            def epilogue_part2(l, sA, h_cur, emn, b0, b1):
                nq = b1 - b0
                nc.vector.tensor_scalar_min(emn[:, 0:nq, :], emn[:, 0:nq, :], 1.0)
                nc.vector.tensor_scalar_max(sA[:, b0:b1, :], sA[:, b0:b1, :], 0.0)
                nc.vector.tensor_tensor(out=sA[:, b0:b1, :], in0=sA[:, b0:b1, :],
                                        in1=emn[:, 0:nq, :], op=OP.add)
                nc.vector.tensor_tensor(out=sA[:, b0:b1, :], in0=sA[:, b0:b1, :],
                                        in1=h_cur[:, b0:b1, :], op=OP.add)
                mean = spool.tile([128, 4], F32, name='mean', tag='mean', bufs=3)
                nc.vector.tensor_reduce(mean[:, 0:nq], sA[:, b0:b1, :],
                                        axis=mybir.AxisListType.X, op=OP.add)
                nc.vector.tensor_scalar_mul(mean[:, 0:nq], mean[:, 0:nq], 1.0 / 96)
                nc.vector.tensor_tensor(out=sA[:, b0:b1, :], in0=sA[:, b0:b1, :],
                                        in1=mean[:, 0:nq].broadcast_to([128, nq, 96]),
                                        op=OP.subtract)
                sq = spool.tile([128, 4, 96], F32, name='sq', tag='sq', bufs=1)
                nc.vector.tensor_tensor(out=sq[:, 0:nq, :], in0=sA[:, b0:b1, :],
                                        in1=sA[:, b0:b1, :], op=OP.mult)
                nc.vector.tensor_reduce(var_all[:, b0:b1], sq[:, 0:nq, :],
                                        axis=mybir.AxisListType.X, op=OP.add)

            def epilogue_quad2(l, sA, h_new, b0, b1):
                nq = b1 - b0
                nc.vector.tensor_tensor(out=sA[:, b0:b1, :], in0=sA[:, b0:b1, :],
                                        in1=rstd_all[:, b0:b1]
                                        .broadcast_to([128, nq, 96]), op=OP.mult)
                nc.vector.tensor_tensor(out=sA[:, b0:b1, :], in0=sA[:, b0:b1, :],
                                        in1=_bmid(lns_t[:, l, :], nq), op=OP.mult)
                nc.vector.tensor_tensor(out=h_new[:, b0:b1, :], in0=sA[:, b0:b1, :],
                                        in1=_bmid(lnb_t[:, l, :], nq), op=OP.add)

            # ---------- layers ----------
            pending_ag = [False]
            for b0, b1 in quad_ranges:
                phaseA_quad(0, h_cur, b0, b1)
            ag_b()

            for l in range(LAYERS):

                # ---- phase B (epilogue + next layer's phase A interleaved
                # per-quad as stream-1 windows complete, so AG_A/AG_B overlap
                # the remaining phase B work) ----
                stg = bpool.tile([128, NBLK, 96], F32, name=f'stg_{l}', tag='stg0')
                den = bpool.tile([128, NBLK, 4], F32, name=f'den_{l}', tag='den0')
                aeL = bpool.tile([128, ncht, 4], BF16, name=f'ae_{l}', tag='aeL',
                                 bufs=2)
                nc.sync.dma_start(aeL[:], ae_v[:, l, :, :])
                h_new = bpool.tile([128, NBLK, 96], F32, name=f'h{l + 1}',
                                   tag='h', bufs=2)
                next_quad = [0]
                pend = []

                def fire_part2():
                    while pend:
                        emn_, a0, a1 = pend.pop(0)
                        epilogue_part2(l, stg, h_cur, emn_, a0, a1)

                def flush_half(hb0, hb1):
                    # one Sqrt per half-layer (per-quad Sqrt thrashes the ACT
                    # table against the Exp used by phase B / ELU)
                    w = hb1 - hb0
                    sdh = spool.tile([128, 32], F32, name='sdh', tag='sdh', bufs=2)
                    nc.scalar.activation(sdh[:, 0:w], var_all[:, hb0:hb1],
                                         AF.Sqrt, bias=eps_t[:, 0:1],
                                         scale=1.0 / 96)
                    nc.vector.reciprocal(rstd_all[:, hb0:hb1], sdh[:, 0:w])
                    for qb0, qb1 in quad_ranges:
                        if qb0 < hb0 or qb1 > hb1:
                            continue
                        epilogue_quad2(l, stg, h_new, qb0, qb1)
                        if l + 1 < LAYERS:
                            phaseA_quad(l + 1, h_new, qb0, qb1)

                def quads_done_through(cb):
                    while (next_quad[0] < len(quad_ranges)
                           and quad_ranges[next_quad[0]][1] <= cb + 1):
                        qb0, qb1 = quad_ranges[next_quad[0]]
                        fire_part2()
                        emn_ = epilogue_part1(l, stg, den, qb0, qb1)
                        pend.append((emn_, qb0, qb1))
                        next_quad[0] += 1
                        if qb1 == SPLIT_BLK:
                            fire_part2()
                            flush_half(0, SPLIT_BLK)

                for s in (0, 1):
                    coff = 0 if s == 0 else nch[0]
                    goff = coff // GRP
                    tbl = tableA if s == 0 else tableB
                    nrows = NTA if s == 0 else NTB
                    cur_ps, cur_b = None, -1
                    for g in range(nch[s] // GRP):
                        cg0 = coff + g * GRP
                        if s == 0 and g == 2 and pending_ag[0]:
                            # AG_B deferred past the first stream-0 gathers so
                            # gpsimd doesn't idle in the collective's input
                            # wait at the layer boundary; tableB is still
                            # ready long before stream 1 starts
                            ag_b()
                            pending_ag[0] = False
                        gt = spool.tile([128, GRP, 128], BF16, name='gt', tag='gt', bufs=5)
                        for hg in range(GRP // 8):
                            nc.gpsimd.dma_gather(
                                gt[:, hg * 8:(hg + 1) * 8, :], tbl[0:nrows, :],
                                idx_all[:, (goff + g) * GRP * 8 + hg * 64:
                                        (goff + g) * GRP * 8 + (hg + 1) * 64],
                                1024, 1024, 128, queue_num=gq[0] % 4)
                            gq[0] += 1
                        mtt = spool.tile([128, GRP, 128], FP8, name='mtt', tag='mtt', bufs=2)
                        nc.sync.dma_start(mtt[:], Mt_v[:, cg0:cg0 + GRP, :])
                        Mb = spool.tile([128, GRP, 128], FP8, name='Mb', tag='Mb', bufs=2)
                        nc.scalar.dma_start(Mb[:], Ms_v[:, cg0:cg0 + GRP, :])

                        adp = pp.tile([128, GRP * 4], F32, name='adp', tag='pD', bufs=2)
                        npad = sum(1 for c in range(GRP) if chunk_win[cg0 + c] < 0)
                        if npad:
                            nc.vector.memset(adp[:, (GRP - npad) * 4:], 0.0)
                        for c in range(GRP):
                            w = chunk_win[cg0 + c]
                            if w < 0:
                                continue
                            nc.tensor.matmul(adp[:, c * 4:(c + 1) * 4], mtt[:, c, :],
                                             adw_t[:, w, :], start=True, stop=True)
                        alpha = spool.tile([128, GRP, 4], F32, name='alpha', tag='alpha', bufs=2)
                        nc.vector.tensor_tensor(
                            out=alpha[:], in0=gt[:, :, 96:100],
                            in1=adp[:].rearrange('p (c f) -> p c f', c=GRP), op=OP.add)
                        nc.vector.tensor_tensor(out=alpha[:], in0=alpha[:],
                                                in1=aeL[:, cg0:cg0 + GRP, :],
                                                op=OP.add)
                        e1 = spool.tile([128, GRP, 4], F32, name='e1', tag='e1', bufs=2)
                        nc.scalar.activation(e1[:], alpha[:], AF.Exp)
                        e2 = spool.tile([128, GRP, 4], F32, name='e2', tag='e2', bufs=2)
                        nc.scalar.activation(e2[:], alpha[:], AF.Exp, scale=NEG)
                        # ex = max(exp(a), exp(.2a)) = exp(leaky(a)), written
                        # straight into gt cols 96:100 (denominator lane)
                        nc.vector.tensor_tensor(out=gt[:, :, 96:100], in0=e1[:],
                                                in1=e2[:], op=OP.max)
                        nc.vector.tensor_tensor(
                            out=gt[:, :, 0:96].rearrange('p c (h r) -> p c h r', h=4),
                            in0=gt[:, :, 0:96].rearrange('p c (h r) -> p c h r', h=4),
                            in1=_bmid(gt[:, :, 96:100], 24, axis=3), op=OP.mult)
                        for c in range(GRP):
                            cb = chunk_win[cg0 + c]
                            if cb < 0:
                                continue
                            if cb != cur_b:
                                assert cur_ps is None
                                cur_ps = pp.tile([128, 104], F32, name='psb', tag='pB', bufs=3)
                                cur_b = cb
                            first = (cg0 + c) == coff + win_ranges[s][cb][0]
                            last = (cg0 + c) == coff + win_ranges[s][cb][1] - 1
                            nc.tensor.matmul(cur_ps[:, 0:100], Mb[:, c, :], gt[:, c, 0:100],
                                             start=first, stop=last)
                            if last:
                                if s == 0:
                                    nc.scalar.activation(stg[:, cb, :],
                                                         cur_ps[:, 0:96], AF.Copy)
                                    nc.scalar.activation(den[:, cb, :],
                                                         cur_ps[:, 96:100], AF.Copy)
                                else:
                                    nc.vector.tensor_tensor(out=stg[:, cb, :],
                                                            in0=stg[:, cb, :],
                                                            in1=cur_ps[:, 0:96],
                                                            op=OP.add)
                                    nc.vector.tensor_tensor(out=den[:, cb, :],
                                                            in0=den[:, cb, :],
                                                            in1=cur_ps[:, 96:100],
                                                            op=OP.add)
                                    quads_done_through(cb)
                                cur_ps, cur_b = None, -1
                    assert cur_ps is None
                assert next_quad[0] == len(quad_ranges)
                fire_part2()
                flush_half(SPLIT_BLK, NBLK)
                if l + 1 < LAYERS:
                    pending_ag[0] = True
                h_cur = h_new

            # ---------- readout ----------
            HB = (NBLK + 1) // 2
            pz = pp.tile([ZCAP, 96], F32, name='pz', tag='pT', bufs=1)
            for hf in range(2):
                hb0, hb1 = hf * HB, min(NBLK, (hf + 1) * HB)
                pbt = bpool.tile([128, HB, ZCAP], F32, name=f'pbt{hf}',
                                 tag='pbt', bufs=2)
                nc.sync.dma_start(
                    pbt[:, 0:hb1 - hb0, :],
                    pball_e[hb0 * 128:hb1 * 128, :].rearrange('(b p) z -> p b z', p=128))
                for b in range(hb0, hb1):
                    nc.tensor.matmul(pz[:], pbt[:, b - hb0, :], h_cur[:, b, :],
                                     start=(b == 0), stop=(b == NBLK - 1))
            zlt = spool.tile([ZCAP, 96], BF16, name='zlt', tag='zlt')
            nc.scalar.activation(zlt[:], pz[:], AF.Copy)
            nc.sync.dma_start(zloc.ap(), zlt[:])
            nc.gpsimd.collective_compute(
                'AllGather', OP.bypass, replica_groups=rg,
                ins=[zloc.ap().opt()], outs=[zall.ap().opt()])

            zat = spool.tile([128, NCORES, 96], BF16, name='zat', tag='zat')
            nc.sync.dma_start(
                zat[:], zall.ap().rearrange('(k p) f -> p k f', p=ZCAP))
            z_T = spool.tile([97, NZC * 128], F32, name='z_T', tag='z_T')
            nc.vector.memset(z_T[96:97, :], 1.0)
            for jc in range(NZC):
                pz2 = pp.tile([128, 96], F32, name='pz2', tag='pA', bufs=1)
                pmt = spool.tile([128, NCORES, 128], BF16, name='pmt', tag='pmt', bufs=1)
                nc.scalar.dma_start(
                    pmt[:], pasm_e[jc * NCORES * 128:(jc + 1) * NCORES * 128, :]
                    .rearrange('(k p) z -> p k z', p=128))
                for k in range(NCORES):
                    nc.tensor.matmul(pz2[:], pmt[:, k, :], zat[:, k, :],
                                     start=(k == 0), stop=(k == NCORES - 1))
                zs = spool.tile([128, 96], F32, name='zs', tag='zs', bufs=2)
                nc.vector.tensor_copy(zs[:], pz2[:])
                ptz = pp.tile([96, 128], F32, name='ptz', tag='pD', bufs=2)
                nc.tensor.transpose(ptz[:], zs[:], ident_t[:])
                nc.vector.tensor_copy(z_T[0:96, jc * 128:(jc + 1) * 128], ptz[:])

            outS = spool.tile([128, NZC, 4], F32, name='outS', tag='outS')
            ooff = 0
            for hi_ in range(3):
                o = head_dims[hi_]
                w1t = spool.tile([97, 96], F32, name='w1t', tag='w1t', bufs=2)
                nc.sync.dma_start(w1t[:], hw[hi_][0][:])
                w2t = spool.tile([97, 48], F32, name='w2t', tag='w2t', bufs=2)
                nc.sync.dma_start(w2t[:], hw[hi_][1][:])
                w3t = spool.tile([48, o], F32, name='w3t', tag='w3t', bufs=2)
                nc.sync.dma_start(w3t[:], hw[hi_][2][0:48, :])
                b3t = spool.tile([4, 1], F32, name='b3t', tag='b3t', bufs=2)
                nc.sync.dma_start(b3t[0:o, :], hw[hi_][2][48:49, 0:o].rearrange('a b -> b a'))
                p1 = pp.tile([96, NZC * 128], F32, name='p1', tag='pT', bufs=1)
                nc.tensor.matmul(p1[:], w1t[:], z_T[:], start=True, stop=True)
                t1 = spool.tile([97, NZC * 128], F32, name='t1', tag='t1', bufs=1)
                nc.scalar.activation(t1[0:96, :], p1[:], AF.Relu)
                nc.vector.memset(t1[96:97, :], 1.0)
                p2 = pp.tile([48, NZC * 128], F32, name='p2', tag='pA', bufs=1)
                nc.tensor.matmul(p2[:], w2t[:], t1[:], start=True, stop=True)
                t2 = spool.tile([48, NZC * 128], F32, name='t2', tag='t2', bufs=1)
                nc.scalar.activation(t2[:], p2[:], AF.Relu)
                p3 = pp.tile([4, NZC * 128], F32, name='p3', tag='pD', bufs=2)
                nc.tensor.matmul(p3[0:o, :], w3t[:], t2[:], start=True, stop=True)
                oh = spool.tile([4, NZC * 128], F32, name='oh', tag='oh', bufs=1)
                nc.vector.tensor_scalar(out=oh[0:o, :], in0=p3[0:o, :],
                                        scalar1=b3t[0:o, 0:1], scalar2=None, op0=OP.add)
                for jc in range(NZC):
                    po = pp.tile([128, 4], F32, name='po', tag='pB', bufs=3)
                    nc.tensor.transpose(po[:, 0:o], oh[0:o, jc * 128:(jc + 1) * 128],
                                        ident_t[0:o, 0:o])
                    nc.vector.tensor_copy(outS[:, jc, ooff:ooff + o], po[:, 0:o])
                ooff += o
            nc.sync.dma_start(
                out_e.ap().rearrange('a z f -> (a z) f')
                    .rearrange('(c p) f -> p c f', p=128), outS[:])

    nc.compile()
    return nc


def _run(inputs, trace=False):
    N = int(np.asarray(inputs['x']).shape[0])
    E = int(np.asarray(inputs['edge_index']).shape[1])
    in_maps, meta = host_prep(inputs, N, E)
    nc = build(meta)
    res = run_bass_kernel_spmd(nc, in_maps, core_ids=list(range(NCORES)), trace=trace)
    return np.asarray(res.results[0]['out'], np.float32).reshape(64, 6, 4), res


def kernel(**inputs):
    return _run(inputs, trace=False)[0]

